# revision 1
# baseline (speedup 1.0000x reference)
"""Multi-head attention (B=4, S=2048, D=1280, H=10, hd=128) on 8 TRN2 NeuronCores.

Sharding: core c handles batch b = c//2 and heads h0 = 5*(c%2) .. h0+5
(data-parallel over batch x head-parallel tensor parallelism). Per core:
  qkvT = wqkv_c^T @ x_b^T           (Q^T, K^T kept head-dim-major; V transposed
                                     to token-major via PE transposes)
  per head: S^T = K_h Q_h^T         (PE; scale folded into exp)
            P = exp(scale * S^T)    (ACT, no max-subtraction: scores are small
                                     because scale = D**-0.5, |s| < ~1)
            colsum via fold-adds (DVE) + ones-matmul (PE)
            O'^T = V_h^T P^T        (PE, PSUM accumulation over j tiles)
            O^T = O'^T / colsum     (reciprocal broadcast via K=1 matmul)
  outT_partial = wout_c^T @ O^T     (row-sharded out projection)
Host: out[b] = outT_{2b}.T + outT_{2b+1}.T + b_out  (all-reduce on host).

Matmul operands are bf16 (weights/activations quantized once on host or at
PSUM->SBUF copy); every accumulation (matmul PSUM, softmax column sums) is
fp32/f32r, so the end-to-end relative error stays ~3e-3.
"""

import numpy as np

B, S, D = 4, 2048, 1280
HEADS = 10
HD = 128
NH = 5              # heads per core
P = 128
SCALE = float(D) ** -0.5
KT_D = D // P       # 10 k-tiles over D
MT = 3 * NH         # 15 m-tiles over local qkv dim (Q 0-4, K 5-9, V 10-14)
NJT = S // P        # 16 j tiles
NIC = S // 512      # 4 i-chunks of 512

_PROGRAM_CACHE = {}


def _build_program(repeat=1):
    if repeat in _PROGRAM_CACHE:
        return _PROGRAM_CACHE[repeat]

    import concourse.mybir as mybir
    from concourse import bacc
    import concourse.tile as tile
    from concourse.masks import make_identity

    F32 = mybir.dt.float32
    F32R = mybir.dt.float32r
    BF16 = mybir.dt.bfloat16
    EXP = mybir.ActivationFunctionType.Exp

    nc = bacc.Bacc()
    xT_d = nc.declare_dram_parameter("xT", [D, S], BF16, isOutput=False)
    wqkv_d = nc.declare_dram_parameter("wqkv", [D, 3 * NH * HD], BF16, isOutput=False)
    wout_d = nc.declare_dram_parameter("wout", [NH * HD, D], BF16, isOutput=False)
    ones_d = nc.declare_dram_parameter("ones_in", [P, 1], F32, isOutput=False)
    onesr_d = nc.declare_dram_parameter("onesr_in", [1, P], F32, isOutput=False)
    out_d = nc.declare_dram_parameter("outT", [D, S], F32, isOutput=True)

    xT_t = xT_d[:].rearrange("(kt p) s -> p kt s", p=P)          # [128, 10, 2048]
    wqkv_t = wqkv_d[:].rearrange("(kt p) m -> p kt m", p=P)      # [128, 10, 1920]
    wout_t = wout_d[:].rearrange("(kt p) m -> p kt m", p=P)      # [128, 5, 1280]

    with tile.TileContext(nc) as tc:
        with (
            tc.tile_pool(name="persist", bufs=1) as persist,
            tc.tile_pool(name="io", bufs=2) as io,
            tc.tile_pool(name="oio", bufs=2) as oio,
            tc.tile_pool(name="work", bufs=4) as work,
            tc.tile_pool(name="ptp", bufs=6) as ptp,
            tc.tile_pool(name="work2", bufs=2) as work2,
            tc.tile_pool(name="ps_mm", bufs=2, space="PSUM") as ps_mm,
            tc.tile_pool(name="ps_acc", bufs=2, space="PSUM") as ps_acc,
            tc.tile_pool(name="ps_one", bufs=1, space="PSUM") as ps_one,
            tc.tile_pool(name="ps_bc", bufs=1, space="PSUM") as ps_bc,
        ):
            QT = persist.tile([P, NH, S], BF16, name="QT")
            KT = persist.tile([P, NH, S], BF16, name="KT")
            V = persist.tile([P, NJT, NH, HD], BF16, name="V")
            WQ = persist.tile([P, KT_D, 3 * NH * HD], BF16, name="WQ")
            WO = persist.tile([P, NH, D], BF16, name="WO")
            ones = persist.tile([P, 1], BF16, name="ones")
            onesr = persist.tile([1, P], F32R, name="onesr")
            ident = persist.tile([P, P], F32, name="ident")

            nc.gpsimd.memset(ones[:], 1.0)
            nc.sync.dma_start(onesr[:], onesr_d[:].bitcast(F32R))
            make_identity(nc, ident[:])

            for rep in range(repeat):
                # ---------------- Phase 1: QKV projection ----------------
                for ic in range(NIC):
                    isl = slice(ic * 512, (ic + 1) * 512)
                    xt = io.tile([P, KT_D, 512], BF16, name="xt")
                    # ACT HWDGE ring: keeps xt loads off the SP ring that
                    # streams the weights at startup
                    nc.scalar.dma_start(xt[:, :5], xT_t[:, :5, isl])
                    nc.scalar.dma_start(xt[:, 5:], xT_t[:, 5:, isl])
                    if rep == 0 and ic == 0:
                        # stream the weights in per-m-tile chunks so the first
                        # matmuls only wait on xt + their own chunk
                        for m in range(MT):
                            nc.sync.dma_start(
                                WQ[:, :, m * P:(m + 1) * P],
                                wqkv_t[:, :, m * P:(m + 1) * P])
                        nc.sync.dma_start(WO[:], wout_t)
                    for m in range(MT):
                        q_ps = ps_mm.tile([P, 1024], F32, name="mm")[:, :512]
                        for kt in range(KT_D):
                            nc.tensor.matmul(
                                q_ps[:], WQ[:, kt, m * P:(m + 1) * P], xt[:, kt, :],
                                start=(kt == 0), stop=(kt == KT_D - 1),
                            )
                        if m < NH:  # Q
                            nc.vector.tensor_copy(QT[:, m, isl], q_ps[:])
                        elif m < 2 * NH:  # K
                            nc.vector.tensor_copy(KT[:, m - NH, isl], q_ps[:])
                        else:  # V: psum holds V^T slice [hd, 512 tokens]
                            h = m - 2 * NH
                            vt = work.tile([P, 512], F32, name="vt")
                            nc.scalar.copy(vt[:], q_ps[:])
                            for tt in range(4):
                                jt = ic * 4 + tt
                                t_ps = ps_bc.tile([P, 512], F32, name="bc")
                                nc.tensor.transpose(
                                    t_ps[:, :P], vt[:, tt * P:(tt + 1) * P], ident[:]
                                )
                                nc.scalar.copy(V[:, jt, h, :], t_ps[:, :P])

                # ------------- Phase 2: attention + out projection -------------
                # Normalize tails run one head late and the out projection one
                # i-chunk late so the PE's in-order stream never waits on the
                # ACT/DVE softmax-sum chain.
                def norm_tail(st):
                    fold, o_ps, OT, h = st
                    sum_ps = ps_one.tile([1, 512], F32, name="one")
                    nc.tensor.matmul(sum_ps, ones[:], fold[:],
                                     start=True, stop=True)
                    s_row = work2.tile([1, 512], F32R, name="s_row")
                    nc.vector.tensor_copy(s_row[:], sum_ps)
                    bc_ps = ps_bc.tile([P, 512], F32, name="bc")
                    nc.tensor.matmul(bc_ps[:], onesr[:], s_row[:],
                                     start=True, stop=True)
                    rec = work2.tile([P, 512], F32, name="rec")
                    nc.vector.reciprocal(rec[:], bc_ps[:])
                    nc.vector.tensor_mul(OT[:, h, :], o_ps[:], rec[:])

                def out_proj(ic, OT, ms, pool=None):
                    isl = slice(ic * 512, (ic + 1) * 512)
                    for m in ms:
                        p_ps = (pool or ps_bc).tile(
                            [P, 512], F32,
                            name="bc" if pool is None else "mm")
                        for kt in range(NH):
                            nc.tensor.matmul(
                                p_ps[:], WO[:, kt, m * P:(m + 1) * P], OT[:, kt, :],
                                start=(kt == 0), stop=(kt == NH - 1),
                            )
                        outc = work.tile([P, 512], F32, name="outc")
                        nc.vector.tensor_copy(outc[:], p_ps[:])
                        nc.sync.dma_start(out_d[m * P:(m + 1) * P, isl], outc[:])

                pending_tail = None
                pending_proj = None
                for ic in range(NIC):
                    isl = slice(ic * 512, (ic + 1) * 512)
                    OT = oio.tile([P, NH, 512], BF16, name="OT")
                    for h in range(NH):
                        fold = work2.tile([P, 512], BF16, name="fold")
                        o_ps = ps_acc.tile([P, 512], F32, name="acc")
                        pt2s = [None] * (NJT // 2)
                        # software-pipelined: the paired S-matmuls + one wide
                        # exp run a pair ahead of the O-matmuls so PE never
                        # waits on ACT.
                        for jp in range(NJT // 2 + 2):
                            if jp < NJT // 2:
                                s_ps = ps_mm.tile([P, 1024], F32, name="mm")
                                for half in range(2):
                                    jt = 2 * jp + half
                                    nc.tensor.matmul(
                                        s_ps[:, half * 512:(half + 1) * 512],
                                        KT[:, h, jt * P:(jt + 1) * P],
                                        QT[:, h, isl], start=True, stop=True,
                                    )
                                pt2 = ptp.tile([P, 1024], BF16, name="pt")
                                nc.scalar.activation(pt2[:], s_ps[:], EXP, scale=SCALE)
                                pt2s[jp] = pt2
                                if jp == 0:
                                    nc.vector.tensor_copy(fold[:], pt2[:, :512])
                                else:
                                    nc.vector.tensor_add(fold[:], fold[:], pt2[:, :512])
                                nc.vector.tensor_add(fold[:], fold[:], pt2[:, 512:])
                            if jp > 1:
                                prev = pt2s[jp - 2]
                                for half in range(2):
                                    jt = 2 * (jp - 2) + half
                                    nc.tensor.matmul(
                                        o_ps[:], V[:, jt, h, :],
                                        prev[:, half * 512:(half + 1) * 512],
                                        start=(jt == 0), stop=(jt == NJT - 1),
                                    )
                            if jp == 1:
                                if pending_tail is not None:
                                    norm_tail(pending_tail)
                                    pending_tail = None
                            if jp in (3, 6) and pending_proj is not None:
                                # spread the (PE-only) out-projection of the
                                # previous i-chunk as fine-grained filler: one
                                # m-group per (head, slot) while ACT catches
                                # up on exps
                                pic, pOT = pending_proj
                                m0 = 2 * h + (0 if jp == 3 else 1)
                                out_proj(pic, pOT, [m0])
                                if h == NH - 1 and jp == 6:
                                    pending_proj = None
                        pending_tail = (fold, o_ps, OT, h)
                    pending_proj = (ic, OT)
                norm_tail(pending_tail)
                out_proj(*pending_proj, range(D // P), pool=ps_mm)

    nc.finalize()
    _PROGRAM_CACHE[repeat] = nc
    return nc


def _shard_inputs(x, w_qkv, w_out):
    """Build the 8 per-core input maps (bf16 operands, host-cast)."""
    import ml_dtypes
    bf16 = ml_dtypes.bfloat16
    ones = np.ones((P, 1), np.float32)
    onesr = np.ones((1, P), np.float32)
    in_maps = []
    for c in range(8):
        b = c // 2
        h0 = NH * (c % 2)
        cols = np.concatenate([
            w_qkv[:, qi * D + h0 * HD: qi * D + (h0 + NH) * HD] for qi in range(3)
        ], axis=1)                                   # [D, 1920]
        in_maps.append(dict(
            xT=np.ascontiguousarray(x[b].T).astype(bf16),          # [D, S]
            wqkv=np.ascontiguousarray(cols).astype(bf16),          # [D, 1920]
            wout=np.ascontiguousarray(
                w_out[h0 * HD:(h0 + NH) * HD, :]).astype(bf16),    # [640, D]
            ones_in=ones,
            onesr_in=onesr,
        ))
    return in_maps


def run_sharded(x, w_qkv, w_out, b_out, repeat=1, trace=False):
    """Run the SPMD program; returns (out [B,S,D], BassKernelResults)."""
    from concourse.bass_utils import run_bass_kernel_spmd

    nc = _build_program(repeat)
    in_maps = _shard_inputs(x, w_qkv, w_out)
    res = run_bass_kernel_spmd(nc, in_maps, list(range(8)), trace=trace)
    out = np.empty((B, S, D), np.float32)
    for b in range(B):
        out[b] = (res.results[2 * b]["outT"].T
                  + res.results[2 * b + 1]["outT"].T
                  + b_out[None, :])
    return out, res


def kernel(x, w_qkv, w_out, b_out):
    x = np.asarray(x, np.float32)
    w_qkv = np.asarray(w_qkv, np.float32)
    w_out = np.asarray(w_out, np.float32)
    b_out = np.asarray(b_out, np.float32)
    out, _ = run_sharded(x, w_qkv, w_out, b_out)
    return out



# revision 47
# speedup vs baseline: 48.2599x; 48.2599x over previous
"""Multi-head attention (B=4, S=2048, D=1280, H=10, hd=128) on 8 TRN2 NeuronCores.

Sharding: core c handles batch b = c//2 and heads h0 = 5*(c%2) .. h0+5
(data-parallel over batch x head-parallel tensor parallelism). Per core:
  qkvT = wqkv_c^T @ x_b^T           (Q^T, K^T kept head-dim-major; V transposed
                                     to token-major via PE transposes)
  per head: S^T = K_h Q_h^T         (PE; scale folded into exp)
            P = exp(scale * S^T)    (ACT, no max-subtraction: scores are small
                                     because scale = D**-0.5, |s| < ~1)
            colsum via fold-adds (DVE) + ones-matmul (PE)
            O'^T = V_h^T P^T        (PE, PSUM accumulation over j tiles)
            O^T = O'^T / colsum     (reciprocal broadcast via K=1 matmul)
  outT_partial = wout_c^T @ O^T     (row-sharded out projection)
Host: out[b] = outT_{2b}.T + outT_{2b+1}.T + b_out  (all-reduce on host).

Matmul operands are bf16 (weights/activations quantized once on host or at
PSUM->SBUF copy); every accumulation (matmul PSUM, softmax column sums) is
fp32/f32r, so the end-to-end relative error stays ~3e-3. The transpose
identity is bf16: the PE transpose runs at 1 cycle/row instead of fp32's 2.
"""

import numpy as np

B, S, D = 4, 2048, 1280
HEADS = 10
HD = 128
NH = 5              # heads per core
P = 128
SCALE = float(D) ** -0.5
KT_D = D // P       # 10 k-tiles over D
MT = 3 * NH         # 15 m-tiles over local qkv dim (Q 0-4, K 5-9, V 10-14)
NJT = S // P        # 16 j tiles
NIC = S // 512      # 4 i-chunks of 512

_PROGRAM_CACHE = {}


def _build_program(repeat=1):
    if repeat in _PROGRAM_CACHE:
        return _PROGRAM_CACHE[repeat]

    import concourse.mybir as mybir
    from concourse import bacc
    import concourse.tile as tile

    F32 = mybir.dt.float32
    F32R = mybir.dt.float32r
    BF16 = mybir.dt.bfloat16
    EXP = mybir.ActivationFunctionType.Exp

    nc = bacc.Bacc()
    xT_d = nc.declare_dram_parameter("xT", [D, S], BF16, isOutput=False)
    wqkv_d = nc.declare_dram_parameter("wqkv", [D, 3 * NH * HD], BF16, isOutput=False)
    wout_d = nc.declare_dram_parameter("wout", [NH * HD, D], BF16, isOutput=False)
    ones_d = nc.declare_dram_parameter("ones_in", [P, 1], F32, isOutput=False)
    onesr_d = nc.declare_dram_parameter("onesr_in", [1, P], F32, isOutput=False)
    ident_d = nc.declare_dram_parameter("ident_in", [P, P], BF16, isOutput=False)
    out_d = nc.declare_dram_parameter("outT", [D, S], F32, isOutput=True)

    xT_t = xT_d[:].rearrange("(kt p) s -> p kt s", p=P)          # [128, 10, 2048]
    wqkv_t = wqkv_d[:].rearrange("(kt p) m -> p kt m", p=P)      # [128, 10, 1920]
    wout_t = wout_d[:].rearrange("(kt p) m -> p kt m", p=P)      # [128, 5, 1280]

    with tile.TileContext(nc) as tc:
        with (
            tc.tile_pool(name="persist", bufs=1) as persist,
            tc.tile_pool(name="io", bufs=2) as io,
            tc.tile_pool(name="oio", bufs=2) as oio,
            tc.tile_pool(name="work", bufs=4) as work,
            tc.tile_pool(name="ptp", bufs=6) as ptp,
            tc.tile_pool(name="work2", bufs=2) as work2,
            tc.tile_pool(name="ps_mm", bufs=2, space="PSUM") as ps_mm,
            tc.tile_pool(name="ps_acc", bufs=2, space="PSUM") as ps_acc,
            tc.tile_pool(name="ps_one", bufs=1, space="PSUM") as ps_one,
            tc.tile_pool(name="ps_bc", bufs=1, space="PSUM") as ps_bc,
        ):
            QT = persist.tile([P, NH, S], BF16, name="QT")
            KT = persist.tile([P, NH, S], BF16, name="KT")
            V = persist.tile([P, NJT, NH, HD], BF16, name="V")
            WQ = persist.tile([P, KT_D, 3 * NH * HD], BF16, name="WQ")
            WO = persist.tile([P, NH, D], BF16, name="WO")
            ones = persist.tile([P, P], BF16, name="ones")
            onesr = persist.tile([1, P], F32R, name="onesr")
            ident = persist.tile([P, P], BF16, name="ident")

            nc.gpsimd.memset(ones[:], 1.0)
            nc.sync.dma_start(onesr[:], onesr_d[:].bitcast(F32R))
            nc.gpsimd.dma_start(ident[:], ident_d[:])

            for rep in range(repeat):
                # ---------------- Phase 1: QKV projection ----------------
                for ic in range(NIC):
                    isl = slice(ic * 512, (ic + 1) * 512)
                    xt = io.tile([P, KT_D, 512], BF16, name="xt")
                    # ACT HWDGE ring: keeps xt loads off the SP ring that
                    # streams the weights at startup
                    nc.scalar.dma_start(xt[:, :5], xT_t[:, :5, isl])
                    nc.scalar.dma_start(xt[:, 5:], xT_t[:, 5:, isl])
                    if rep == 0 and ic == 0:
                        # stream the weights in per-m-tile chunks so the first
                        # matmuls only wait on xt + their own chunk
                        for m in range(MT):
                            nc.sync.dma_start(
                                WQ[:, :, m * P:(m + 1) * P],
                                wqkv_t[:, :, m * P:(m + 1) * P])
                        nc.sync.dma_start(WO[:], wout_t)
                    for m in range(MT):
                        q_ps = ps_mm.tile([P, 1024], F32, name="mm")[:, :512]
                        for kt in range(KT_D):
                            nc.tensor.matmul(
                                q_ps[:], WQ[:, kt, m * P:(m + 1) * P], xt[:, kt, :],
                                start=(kt == 0), stop=(kt == KT_D - 1),
                            )
                        if m < NH:  # Q
                            nc.vector.tensor_copy(QT[:, m, isl], q_ps[:])
                        elif m < 2 * NH:  # K
                            nc.vector.tensor_copy(KT[:, m - NH, isl], q_ps[:])
                        else:  # V: psum holds V^T slice [hd, 512 tokens]
                            h = m - 2 * NH
                            vt = work.tile([P, 512], BF16, name="vt")
                            nc.scalar.copy(vt[:], q_ps[:])
                            for tt in range(4):
                                jt = ic * 4 + tt
                                t_ps = ps_bc.tile([P, 512], BF16, name="bc")
                                nc.tensor.transpose(
                                    t_ps[:, :P], vt[:, tt * P:(tt + 1) * P], ident[:]
                                )
                                nc.scalar.copy(V[:, jt, h, :], t_ps[:, :P])

                # ------------- Phase 2: attention + out projection -------------
                # Normalize tails run one head late and the out projection one
                # i-chunk late so the PE's in-order stream never waits on the
                # ACT/DVE softmax-sum chain.
                def norm_tail(st):
                    # ones is [P,P]: the colsum matmul writes the softmax
                    # denominator pre-broadcast to all 128 partitions, so no
                    # s_row copy / K=1 broadcast matmul is needed.
                    fold, o_ps, OT, h = st
                    sum_ps = ps_one.tile([P, 512], F32, name="one")
                    nc.tensor.matmul(sum_ps[:], ones[:], fold[:],
                                     start=True, stop=True)
                    rec = work2.tile([P, 512], F32, name="rec")
                    nc.vector.reciprocal(rec[:], sum_ps[:])
                    nc.vector.tensor_mul(OT[:, h, :], o_ps[:], rec[:])

                def out_proj(ic, OT, ms, pool=None):
                    isl = slice(ic * 512, (ic + 1) * 512)
                    for m in ms:
                        p_ps = (pool or ps_bc).tile(
                            [P, 512], F32,
                            name="bc" if pool is None else "mm")
                        for kt in range(NH):
                            nc.tensor.matmul(
                                p_ps[:], WO[:, kt, m * P:(m + 1) * P], OT[:, kt, :],
                                start=(kt == 0), stop=(kt == NH - 1),
                            )
                        outc = work.tile([P, 512], F32, name="outc")
                        nc.vector.tensor_copy(outc[:], p_ps[:])
                        nc.sync.dma_start(out_d[m * P:(m + 1) * P, isl], outc[:])

                pending_tail = None
                pending_proj = None
                for ic in range(NIC):
                    isl = slice(ic * 512, (ic + 1) * 512)
                    OT = oio.tile([P, NH, 512], BF16, name="OT")
                    for h in range(NH):
                        fold = work2.tile([P, 512], BF16, name="fold")
                        o_ps = ps_acc.tile([P, 512], F32, name="acc")
                        pt2s = [None] * (NJT // 2)
                        # software-pipelined: the paired S-matmuls + one wide
                        # exp run a pair ahead of the O-matmuls so PE never
                        # waits on ACT.
                        for jp in range(NJT // 2 + 2):
                            if jp < NJT // 2:
                                s_ps = ps_mm.tile([P, 1024], F32, name="mm")
                                for half in range(2):
                                    jt = 2 * jp + half
                                    nc.tensor.matmul(
                                        s_ps[:, half * 512:(half + 1) * 512],
                                        KT[:, h, jt * P:(jt + 1) * P],
                                        QT[:, h, isl], start=True, stop=True,
                                    )
                                pt2 = ptp.tile([P, 1024], BF16, name="pt")
                                nc.scalar.activation(pt2[:], s_ps[:], EXP, scale=SCALE)
                                pt2s[jp] = pt2
                                if jp == 0:
                                    nc.vector.tensor_copy(fold[:], pt2[:, :512])
                                else:
                                    nc.vector.tensor_add(fold[:], fold[:], pt2[:, :512])
                                nc.vector.tensor_add(fold[:], fold[:], pt2[:, 512:])
                            if jp > 1:
                                prev = pt2s[jp - 2]
                                for half in range(2):
                                    jt = 2 * (jp - 2) + half
                                    nc.tensor.matmul(
                                        o_ps[:], V[:, jt, h, :],
                                        prev[:, half * 512:(half + 1) * 512],
                                        start=(jt == 0), stop=(jt == NJT - 1),
                                    )
                            if jp == 1:
                                if pending_tail is not None:
                                    norm_tail(pending_tail)
                                    pending_tail = None
                            if jp in (3, 6) and pending_proj is not None:
                                # spread the (PE-only) out-projection of the
                                # previous i-chunk as fine-grained filler: one
                                # m-group per (head, slot) while ACT catches
                                # up on exps
                                pic, pOT = pending_proj
                                m0 = 2 * h + (0 if jp == 3 else 1)
                                out_proj(pic, pOT, [m0])
                                if h == NH - 1 and jp == 6:
                                    pending_proj = None
                        pending_tail = (fold, o_ps, OT, h)
                    pending_proj = (ic, OT)
                norm_tail(pending_tail)
                out_proj(*pending_proj, range(D // P), pool=ps_mm)

    nc.finalize()
    _PROGRAM_CACHE[repeat] = nc
    return nc


def _shard_inputs(x, w_qkv, w_out):
    """Build the 8 per-core input maps (bf16 operands, host-cast)."""
    import ml_dtypes
    bf16 = ml_dtypes.bfloat16
    ones = np.ones((P, 1), np.float32)
    onesr = np.ones((1, P), np.float32)
    ident = np.eye(P, dtype=bf16)
    in_maps = []
    for c in range(8):
        b = c // 2
        h0 = NH * (c % 2)
        cols = np.concatenate([
            w_qkv[:, qi * D + h0 * HD: qi * D + (h0 + NH) * HD] for qi in range(3)
        ], axis=1)                                   # [D, 1920]
        in_maps.append(dict(
            xT=np.ascontiguousarray(x[b].T).astype(bf16),          # [D, S]
            wqkv=np.ascontiguousarray(cols).astype(bf16),          # [D, 1920]
            wout=np.ascontiguousarray(
                w_out[h0 * HD:(h0 + NH) * HD, :]).astype(bf16),    # [640, D]
            ones_in=ones,
            onesr_in=onesr,
            ident_in=ident,
        ))
    return in_maps


def run_sharded(x, w_qkv, w_out, b_out, repeat=1, trace=False):
    """Run the SPMD program; returns (out [B,S,D], BassKernelResults)."""
    from concourse.bass_utils import run_bass_kernel_spmd

    nc = _build_program(repeat)
    in_maps = _shard_inputs(x, w_qkv, w_out)
    res = run_bass_kernel_spmd(nc, in_maps, list(range(8)), trace=trace)
    out = np.empty((B, S, D), np.float32)
    for b in range(B):
        out[b] = (res.results[2 * b]["outT"].T
                  + res.results[2 * b + 1]["outT"].T
                  + b_out[None, :])
    return out, res


def kernel(x, w_qkv, w_out, b_out):
    x = np.asarray(x, np.float32)
    w_qkv = np.asarray(w_qkv, np.float32)
    w_out = np.asarray(w_out, np.float32)
    b_out = np.asarray(b_out, np.float32)
    out, _ = run_sharded(x, w_qkv, w_out, b_out)
    return out
